# revision 10
# baseline (speedup 1.0000x reference)
"""Bahdanau-attention kernel for TRN2, data-parallel over 8 NeuronCores.

Math: the reference applies softmax over the LAST axis of scores, which has
size 1 — softmax over a singleton axis is identically 1.0 (exp(x-x)/exp(x-x)).
Therefore:
    attn_weights = ones(bs, sq, 21, 7, 1)
    attn_out     = attn_weights * keys = broadcast(keys, (bs, sq, 21, 7, 256))
independent of queries / masks / all projection weights. The kernel is a pure
DMA problem: per core, read its keys shard into SBUF and write it back 21x
(broadcast over the query axis), plus a ones fill for the weights output.

Per-core traffic: read 1.9 MB + write 38.7 MB at the ~400 GB/s measured SDMA
aggregate => ~101 us steady state. The input load is chunked (4 x 448 KB) and
interleaved with the output writes so the first output DMA starts ~1.5 us
after the first chunk lands rather than after the full load.
"""

import numpy as np

from concourse import bass, mybir
from concourse.bass_utils import run_bass_kernel_spmd

BS, SQ, NQ, NK, D = 16, 128, 21, 7, 256
N_CORES = 8
BPC = BS // N_CORES  # batches per core
ROW = NK * D  # contiguous floats per (b, s): 1792
W_ROW = NQ * NK  # attn_weights floats per (b, s): 147
PCH = 64  # partitions per pipeline chunk


def _build() -> bass.Bass:
    nc = bass.Bass()
    keys_in = nc.declare_dram_parameter(
        "keys", [BPC, SQ, ROW], mybir.dt.float32, isOutput=False
    )
    ones_in = nc.declare_dram_parameter(
        "ones", [SQ, W_ROW], mybir.dt.float32, isOutput=False
    )
    attn_out = nc.declare_dram_parameter(
        "attn_out", [BPC, SQ, NQ, ROW], mybir.dt.float32, isOutput=True
    )
    attn_w = nc.declare_dram_parameter(
        "attn_w", [BPC, SQ, W_ROW], mybir.dt.float32, isOutput=True
    )

    with (
        nc.Block() as block,
        nc.semaphore("in_sem") as in_sem,
        nc.semaphore("out_sem") as out_sem,
        nc.sbuf_tensor("kt", [SQ, BPC, ROW], mybir.dt.float32) as kt,
    ):

        QH = NQ // 2  # q copies 0..QH-1 issued on SP ring, QH..NQ-1 on ACT ring

        @block.sync
        def _(sync: bass.BassEngine):
            # keys shard -> SBUF, one full-width (128-partition) DMA per batch
            # (sub-128-partition DMAs run at half bandwidth)
            for b in range(BPC):
                sync.dma_start(out=kt[:, b], in_=keys_in[b]).then_inc(in_sem, 16)
            # weights while the load is in flight: DRAM->DRAM ones broadcast
            sync.dma_start(
                out=attn_w[:].transpose([1, 0, 2]),
                in_=ones_in[:].unsqueeze(1).broadcast_to((SQ, BPC, W_ROW)),
            ).then_inc(out_sem, 16)
            # broadcast writes: batch b starts as soon as its load landed
            for b in range(BPC):
                sync.wait_ge(in_sem, 16 * (b + 1))
                sync.dma_start(
                    out=attn_out[b, :, :QH],
                    in_=kt[:, b].unsqueeze(1).broadcast_to((SQ, QH, ROW)),
                ).then_inc(out_sem, 16)
            # 1 w + 2 half-outs here + 2 half-outs on scalar
            sync.wait_ge(out_sem, 16 * 5)

        @block.scalar
        def _(scalar: bass.BassEngine):
            for b in range(BPC):
                scalar.wait_ge(in_sem, 16 * (b + 1))
                scalar.dma_start(
                    out=attn_out[b, :, QH:],
                    in_=kt[:, b].unsqueeze(1).broadcast_to((SQ, NQ - QH, ROW)),
                ).then_inc(out_sem, 16)

    return nc


_NC_CACHE: list = []


def kernel(**inputs: np.ndarray):
    keys = np.ascontiguousarray(
        np.asarray(inputs["keys"], dtype=np.float32).reshape(BS, SQ, ROW)
    )
    if not _NC_CACHE:
        _NC_CACHE.append(_build())
    nc = _NC_CACHE[0]

    ones = np.ones((SQ, W_ROW), dtype=np.float32)
    in_maps = [
        {"keys": keys[c * BPC : (c + 1) * BPC], "ones": ones} for c in range(N_CORES)
    ]
    res = run_bass_kernel_spmd(nc, in_maps, core_ids=list(range(N_CORES)))

    attn_out = np.concatenate(
        [r["attn_out"].reshape(BPC, SQ, NQ, NK, D) for r in res.results], axis=0
    )
    attn_w = np.concatenate(
        [r["attn_w"].reshape(BPC, SQ, NQ, NK, 1) for r in res.results], axis=0
    )
    return attn_out, attn_w


# revision 11
# speedup vs baseline: 1.2101x; 1.2101x over previous
"""Bahdanau-attention kernel for TRN2, data-parallel over 8 NeuronCores.

Math: the reference applies softmax over the LAST axis of scores, which has
size 1 — softmax over a singleton axis is identically 1.0 (exp(x-x)/exp(x-x)).
Therefore:
    attn_weights = ones(bs, sq, 21, 7, 1)
    attn_out     = attn_weights * keys = broadcast(keys, (bs, sq, 21, 7, 256))
independent of queries / masks / all projection weights. The kernel is a pure
DMA problem: per core, read its keys shard into SBUF and write it back 21x
(broadcast over the query axis), plus a ones fill for the weights output.

Per-core traffic: read 1.9 MB + write 38.7 MB at the ~415 GB/s measured SDMA
aggregate => ~97 us dense phase + ~8.5 us fixed preamble/first-load latency +
~2 us completion tail (~108 us total on a quiet core). All DMAs span the full
128 partitions (sub-128-partition DMAs run at half bandwidth). The input load
is split per batch so batch 0's output starts while batch 1 still loads, the
weights fill is DRAM->DRAM during the load window, and the output copies are
split between the two HWDGE rings (SP via nc.sync, ACT via nc.scalar) for
parallel descriptor generation.
"""

import numpy as np

from concourse import bass, mybir
from concourse.bass_utils import run_bass_kernel_spmd

BS, SQ, NQ, NK, D = 16, 128, 21, 7, 256
N_CORES = 8
BPC = BS // N_CORES  # batches per core
ROW = NK * D  # contiguous floats per (b, s): 1792
W_ROW = NQ * NK  # attn_weights floats per (b, s): 147
PCH = 64  # partitions per pipeline chunk


def _build() -> bass.Bass:
    nc = bass.Bass()
    keys_in = nc.declare_dram_parameter(
        "keys", [BPC, SQ, ROW], mybir.dt.float32, isOutput=False
    )
    ones_in = nc.declare_dram_parameter(
        "ones", [SQ, W_ROW], mybir.dt.float32, isOutput=False
    )
    attn_out = nc.declare_dram_parameter(
        "attn_out", [BPC, SQ, NQ, ROW], mybir.dt.float32, isOutput=True
    )
    attn_w = nc.declare_dram_parameter(
        "attn_w", [BPC, SQ, W_ROW], mybir.dt.float32, isOutput=True
    )

    with (
        nc.Block() as block,
        nc.semaphore("in_sem") as in_sem,
        nc.semaphore("out_sem") as out_sem,
        nc.sbuf_tensor("kt", [SQ, BPC, ROW], mybir.dt.float32) as kt,
    ):

        QH = NQ // 2  # q copies 0..QH-1 issued on SP ring, QH..NQ-1 on ACT ring

        @block.sync
        def _(sync: bass.BassEngine):
            # keys shard -> SBUF, one full-width (128-partition) DMA per batch
            # (sub-128-partition DMAs run at half bandwidth)
            for b in range(BPC):
                sync.dma_start(out=kt[:, b], in_=keys_in[b]).then_inc(in_sem, 16)
            # weights while the load is in flight: DRAM->DRAM ones broadcast
            sync.dma_start(
                out=attn_w[:].transpose([1, 0, 2]),
                in_=ones_in[:].unsqueeze(1).broadcast_to((SQ, BPC, W_ROW)),
            ).then_inc(out_sem, 16)
            # broadcast writes: batch b starts as soon as its load landed
            for b in range(BPC):
                sync.wait_ge(in_sem, 16 * (b + 1))
                sync.dma_start(
                    out=attn_out[b, :, :QH],
                    in_=kt[:, b].unsqueeze(1).broadcast_to((SQ, QH, ROW)),
                ).then_inc(out_sem, 16)
            # 1 w + 2 half-outs here + 2 half-outs on scalar
            sync.wait_ge(out_sem, 16 * 5)

        @block.scalar
        def _(scalar: bass.BassEngine):
            for b in range(BPC):
                scalar.wait_ge(in_sem, 16 * (b + 1))
                scalar.dma_start(
                    out=attn_out[b, :, QH:],
                    in_=kt[:, b].unsqueeze(1).broadcast_to((SQ, NQ - QH, ROW)),
                ).then_inc(out_sem, 16)

    return nc


_NC_CACHE: list = []


def kernel(**inputs: np.ndarray):
    keys = np.ascontiguousarray(
        np.asarray(inputs["keys"], dtype=np.float32).reshape(BS, SQ, ROW)
    )
    if not _NC_CACHE:
        _NC_CACHE.append(_build())
    nc = _NC_CACHE[0]

    ones = np.ones((SQ, W_ROW), dtype=np.float32)
    in_maps = [
        {"keys": keys[c * BPC : (c + 1) * BPC], "ones": ones} for c in range(N_CORES)
    ]
    res = run_bass_kernel_spmd(nc, in_maps, core_ids=list(range(N_CORES)))

    attn_out = np.concatenate(
        [r["attn_out"].reshape(BPC, SQ, NQ, NK, D) for r in res.results], axis=0
    )
    attn_w = np.concatenate(
        [r["attn_w"].reshape(BPC, SQ, NQ, NK, 1) for r in res.results], axis=0
    )
    return attn_out, attn_w
